# revision 7
# baseline (speedup 1.0000x reference)
"""Locally-connected autoencoder (128 independent 256->8->256 per-patch linears
+ sigmoid) on 8 Trainium2 NeuronCores.

Strategy (v2: all-fp16)
-----------------------
Feature-parallel: core k owns bands (2k, 2k+1) for all 2048 samples, where a
"band" is one row of 8 patches (16 image rows = 2048 contiguous features).
Weights are tiny (~1 MB) so each core keeps its 2 bands' block-diagonal
weights resident in SBUF.

The 2e-2 rel-err budget is ~400x looser than the old fp32-compensated scheme
needed, so everything rides in fp16 (10 mantissa bits): x, We, z, Wd, and the
sigmoid output are all fp16 (measured max rel err ~4.8e-3).  That halves HBM
traffic versus the bf16 hi/lo scheme (2 B/elem in + 2 B/elem out = ~35 MB/core
vs ~69) and halves the matmul count (no compensation passes): 256 matmuls of
[128x{64,128}] @ [128,512] per core -- both encode and decode stream the
minimum number of PE passes.

Per band (w-major x layout [W=128 part, r, n] shipped by host):
  encode:  z[64(pw,h), n512] = sum_r Wenc_bd[r].T @ X[:, r, n512]   (16 mm)
  bias:    ACT copies z PSUM->SBUF fp16 adding be as per-partition bias
  decode:  out^T[128 w', n512] = Wdec_bd[:, fc].T @ z               (K=64)
  sigmoid: ACT reads decode PSUM (1024-wide, 2 banks) -> fp16 out^T tiles,
           batched 4 image-rows per DMA (16 KB/partition descriptors).
The host re-transposes the per-core out^T back to [n, f] and upcasts.
"""

import numpy as np

# problem constants (hardcoded per contract)
H, W, PS = 256, 128, 16
NPH, NPW = H // PS, W // PS      # 16 bands, 8 patches/band
P, D, HID = NPH * NPW, PS * PS, 8
NSMP = 4 * 512                    # 2048 samples
BANDW = PS * W                    # 2048 features per band
NCORES = 8
BPC = NPH // NCORES               # 2 bands per core
M = NPW * HID                     # 64 latent rows per band
FCB = 4                           # image rows per out DMA batch

_PROG = None
LAST_EXEC_NS = None   # filled when kernel() runs with _trace=True


def _install_ntff_hook():
    """The agent image's antenv lacks axon_hooks; synthesize it so
    run_bass_kernel_spmd(trace=True) can capture NTFF profiles."""
    import sys, types
    try:
        import antenv.axon_hooks  # noqa: F401
        return
    except ImportError:
        pass
    try:
        from trn_agent_boot.trn_boot import _ntff_profile_via_ctypes
        hook = _ntff_profile_via_ctypes('/opt/axon/libaxon_pjrt.so')
    except Exception:
        hook = None
    import antenv
    mod = types.ModuleType("antenv.axon_hooks")
    mod.get_axon_ntff_profile_hook = lambda: hook
    mod.set_axon_ntff_profile_hook = lambda h: None
    antenv.axon_hooks = mod
    sys.modules["antenv.axon_hooks"] = mod


def _patch_tile_drain():
    """This image's walrus caps instructions at ONE sync wait.  Tile attaches
    one wait per outstanding semaphore to the exit drain and can give body
    instructions several waits.  Split: hoist all but one wait onto fresh
    single-wait NOPs inserted immediately before, on the same engine (engine
    streams are in-order, so this is semantics-preserving)."""
    import concourse.tile as tile
    import bass_rust
    from concourse.vector_clock import ScopedClock

    if getattr(tile.TileContext, "_drain_split_patched", False):
        return

    def patched(self, tick_clock, wait_clock):
        drain_inst = self.nc.sync.drain()
        wait_clock.add_sem_waits(
            drain_inst.ins, ScopedClock({None: tick_clock.global_clock})
        )
        si = drain_inst.ins.sync_info
        w = si.on_wait if si else []
        if len(w) > 1:
            drain_inst.ins.sync_info.on_wait = w[:1]
            for x in w[1:]:
                d2 = self.nc.sync.drain()
                d2.ins.sync_info = bass_rust.SyncInfo(on_wait=[x], on_update=[])
        self.nc.all_engine_barrier()
        assert self.sems is not None
        popped = self.nc._tile_sem_poison_stack.pop()
        assert popped is self._sem_poison
        self.nc.clear_and_free_semaphores(list(self.sems.allocated().values()))
        self.nc.all_engine_barrier()

    tile.TileContext._drain_and_barrier = patched

    from concourse import mybir
    from concourse.tile_scheduler import BassTileLoopBlock, BassTileRelease

    _special = [BassTileLoopBlock, BassTileRelease]
    for nm in ("BassTileCriticalSection", "BassTileBranchHintPlaceholder",
               "TileBranchInst", "BassTileConditionalBlock"):
        cls = getattr(tile, nm, None)
        if cls is not None:
            _special.append(cls)
    _special = tuple(_special)

    orig_lower = tile.TileContext._lower_ordered_insts

    def patched_lower(self, ordered):
        for bb_name in list(ordered.keys()):
            insts = ordered[bb_name]
            if not any(
                i.sync_info is not None and len(i.sync_info.on_wait) > 1
                for i in insts
            ):
                continue
            new = []
            for inst in insts:
                si = inst.sync_info
                if (
                    si is not None
                    and len(si.on_wait) > 1
                    and not isinstance(inst, _special)
                ):
                    waits = list(si.on_wait)
                    for x in waits[:-1]:
                        nop = mybir.InstNoOp(
                            name=self.nc.get_next_instruction_name(),
                            ins=[],
                            outs=[],
                            engine=inst.engine,
                            bass_nofuse=True,
                            sync_info=bass_rust.SyncInfo(on_wait=[x], on_update=[]),
                        )
                        new.append(nop)
                    si.on_wait = waits[-1:]
                new.append(inst)
            ordered[bb_name] = new
        return orig_lower(self, ordered)

    tile.TileContext._lower_ordered_insts = patched_lower
    tile.TileContext._drain_split_patched = True


def _build_program():
    """Build the per-core Bass program (same program for all 8 cores)."""
    global _PROG
    if _PROG is not None:
        return _PROG

    import concourse.bass as bass
    import concourse.tile as tile
    from concourse import mybir

    _patch_tile_drain()

    f32 = mybir.dt.float32
    f16 = mybir.dt.float16
    AFT = mybir.ActivationFunctionType

    nc = bass.Bass("TRN2", target_bir_lowering=False, debug=False)

    # All DRAM layouts match their SBUF destination order => every DMA is a
    # contiguous src->dst copy with large per-partition descriptors.
    xt_d = nc.dram_tensor("xt", [BPC, W, PS, NSMP], f16, kind="ExternalInput").ap()
    we_d = nc.dram_tensor("we", [W, BPC, PS, M], f16, kind="ExternalInput").ap()
    wd_d = nc.dram_tensor("wd", [M, BPC, BANDW], f16, kind="ExternalInput").ap()
    bev_d = nc.dram_tensor("bev", [M, BPC], f32, kind="ExternalInput").ap()
    bdv_d = nc.dram_tensor("bdv", [W, BPC, PS], f32, kind="ExternalInput").ap()
    out_d = nc.dram_tensor("out", [BPC, W, PS, NSMP], f16, kind="ExternalOutput").ap()

    with tile.TileContext(nc) as tc:
        with (
            tc.tile_pool(name="singles", bufs=1) as singles,
            tc.tile_pool(name="xp", bufs=4) as xpool,
            tc.tile_pool(name="zhp", bufs=6) as zhpool,
            tc.tile_pool(name="outsb", bufs=5) as opool,
            tc.tile_pool(name="zps", bufs=2, space="PSUM") as zpsum,
            tc.tile_pool(name="ops", bufs=3, space="PSUM") as opsum,
        ):
            # encode weights first: the first matmul needs only these
            we_sb = singles.tile([W, BPC, PS, M], f16)
            nc.sync.dma_start(out=we_sb, in_=we_d)

            # x tiles: full 16 rows x 1024 samples (4.2 MB, 2 KB descriptors).
            # ALL x DMAs issue up-front so in-descriptors sit ahead of every
            # out-descriptor in the FIFO DMA queues -- a late x tile queueing
            # behind the out backlog starves the whole pipeline.
            x_tiles, zh_tiles = {}, {}

            def load_x(b, h):
                Xt = xpool.tile([W, PS, 1024], f16, name="Xt")
                nc.sync.dma_start(out=Xt, in_=xt_d[b, :, :, 1024 * h:1024 * (h + 1)])
                x_tiles[(b, h)] = Xt

            load_x(0, 0)
            bev_sb = singles.tile([M, BPC], f32)
            nc.sync.dma_start(out=bev_sb, in_=bev_d)
            load_x(0, 1)
            wd_sb = singles.tile([M, BPC, BANDW], f16)
            nc.sync.dma_start(out=wd_sb, in_=wd_d)
            bdv_sb = singles.tile([W, BPC, PS], f32)
            nc.sync.dma_start(out=bdv_sb, in_=bdv_d)
            load_x(1, 0)
            load_x(1, 1)

            def enc(b, c):
                """16 matmuls -> z_ps; DVE adds bias + converts to fp16."""
                Xt = x_tiles[(b, c // 2)]
                s0 = 512 * (c % 2)
                z_ps = zpsum.tile([M, 512], f32, name="z_ps")
                for r in range(PS):
                    nc.tensor.matmul(
                        z_ps, lhsT=we_sb[:, b, r, :],
                        rhs=Xt[:, r, s0:s0 + 512],
                        start=(r == 0), stop=(r == PS - 1),
                    )
                zh = zhpool.tile([M, 512], f16, name="zh")
                nc.vector.tensor_scalar_add(zh, z_ps, bev_sb[:, b:b + 1])
                zh_tiles[(b, c)] = zh

            def dec(b, j2, fg):
                """4 image rows x 1024 samples: 8 mm + 4 sigmoid + 1 DMA."""
                za, zb = zh_tiles[(b, 2 * j2)], zh_tiles[(b, 2 * j2 + 1)]
                o_sb = opool.tile([W, FCB, 1024], f16, name="o_sb")
                for fi in range(FCB):
                    fc = FCB * fg + fi
                    wstat = wd_sb[:, b, fc * W:(fc + 1) * W]
                    o_ps = opsum.tile([W, 1024], f32, name="o_ps")
                    nc.tensor.matmul(o_ps[:, 0:512], lhsT=wstat, rhs=za,
                                     start=True, stop=True)
                    nc.tensor.matmul(o_ps[:, 512:1024], lhsT=wstat, rhs=zb,
                                     start=True, stop=True)
                    nc.scalar.activation(
                        out=o_sb[:, fi, :], in_=o_ps, func=AFT.Sigmoid,
                        bias=bdv_sb[:, b, fc:fc + 1], scale=1.0,
                    )
                nc.scalar.dma_start(
                    out=out_d[b, :, FCB * fg:FCB * (fg + 1),
                              1024 * j2:1024 * (j2 + 1)],
                    in_=o_sb)

            # software-pipelined schedule: encode chunks woven into the
            # ACT-paced decode stream keep the PE continuously busy (clock
            # stays ramped) while ACT runs gapless from ~15us on.
            enc(0, 0); enc(0, 1)
            dec(0, 0, 0); enc(0, 2); dec(0, 0, 1); enc(0, 3)
            dec(0, 0, 2); dec(0, 0, 3)
            dec(0, 1, 0); enc(1, 0); dec(0, 1, 1); enc(1, 1)
            dec(0, 1, 2); dec(0, 1, 3)
            dec(1, 0, 0); enc(1, 2); dec(1, 0, 1); enc(1, 3)
            dec(1, 0, 2); dec(1, 0, 3)
            dec(1, 1, 0); dec(1, 1, 1); dec(1, 1, 2); dec(1, 1, 3)

    _PROG = nc
    return nc


def _host_prep(x, We, be, Wd, bd):
    """Slice/transpose inputs into per-core maps (pure numpy)."""
    x = np.ascontiguousarray(np.asarray(x, dtype=np.float32)).reshape(NSMP, H * W)
    We = np.asarray(We, dtype=np.float32)
    be = np.asarray(be, dtype=np.float32)
    Wd = np.asarray(Wd, dtype=np.float32)
    bd = np.asarray(bd, dtype=np.float32)

    # x^T as [band, w, r, n] fp16
    xT = x.T.reshape(NPH, PS, W, NSMP)                   # [band, r, w, n]
    xt = np.ascontiguousarray(
        xT.transpose(0, 2, 1, 3), dtype=np.float16)      # [band, w, r, n]

    # encode block-diag: wenc[ph, r, 16pw+c, 8pw+h] = We[ph*8+pw, h, r*16+c]
    We6 = We.reshape(NPH, NPW, HID, PS, PS)              # [ph, pw, h, r, c]
    wenc = np.zeros((NPH, PS, W, M), dtype=np.float16)
    for pw in range(NPW):
        wenc[:, :, PS * pw:PS * (pw + 1), HID * pw:HID * (pw + 1)] = (
            We6[:, pw].transpose(0, 2, 3, 1)             # [ph, r, c, h]
        )

    # decode rhs: wdec[ph, 8pw+h, 128r'+16pw+c'] = Wd[ph*8+pw, r'*16+c', h]
    Wd5 = Wd.reshape(NPH, NPW, PS, PS, HID)              # [ph, pw, r', c', h]
    wdec = np.zeros((NPH, M, BANDW), dtype=np.float16)
    wdec_v = wdec.reshape(NPH, NPW, HID, PS, NPW, PS)
    for pw in range(NPW):
        wdec_v[:, pw, :, :, pw, :] = Wd5[:, pw].transpose(0, 3, 1, 2)  # [ph, h, r', c']

    bev = be.reshape(NPH, M)                             # [ph, 64]
    # decode bias, per-partition for the transposed output: bdv[w', ph, r']
    bd4 = bd.reshape(NPH, NPW, PS, PS)                   # [ph, pw, r', c']
    bdv = bd4.transpose(1, 3, 0, 2).reshape(W, NPH, PS)  # [16pw+c', ph, r']

    in_maps = []
    for k in range(NCORES):
        sl = slice(BPC * k, BPC * (k + 1))
        in_maps.append({
            "xt": np.ascontiguousarray(xt[sl]),
            "we": np.ascontiguousarray(wenc[sl].transpose(2, 0, 1, 3)),  # [w,b,r,m]
            "wd": np.ascontiguousarray(wdec[sl].transpose(1, 0, 2)),     # [m,b,f]
            "bev": np.ascontiguousarray(bev[sl].T),
            "bdv": np.ascontiguousarray(bdv[:, sl, :]),
        })
    return in_maps


def kernel(x, We, be, Wd, bd, _trace=False):
    global LAST_EXEC_NS
    from concourse.bass_utils import run_bass_kernel_spmd

    if _trace:
        _install_ntff_hook()

    nc = _build_program()
    in_maps = _host_prep(x, We, be, Wd, bd)
    res = run_bass_kernel_spmd(nc, in_maps, list(range(NCORES)), trace=_trace)
    if _trace:
        LAST_EXEC_NS = res.exec_time_ns

    # out_k is out^T: [band, w', fc, n] -> out[n, band*2048 + 128 fc + w']
    o = np.stack([res.results[k]["out"] for k in range(NCORES)])
    o = o.reshape(NPH, W, PS, NSMP)                      # [band, w', fc, n]
    out = o.transpose(3, 0, 2, 1).reshape(NSMP, H * W)   # [n, f]
    return np.ascontiguousarray(out.reshape(4, 512, H * W), dtype=np.float32)
